# revision 8
# baseline (speedup 1.0000x reference)
"""Trainium2 Bass kernel for the ATTR_TRANSFORMER pooling module.

Computation (per batch row b):
    attn_x  = tanh(x[b] @ W_attr.T + b_attr)            # [S, H]
    ru      = (user @ W_user.T + b_user)[idx[b]]        # [H]
    score   = (attn_x * mask[b,:,None]) @ ru            # [S]
    w       = softmax(score)                            # [S]
    out_x   = attn_x.T @ (w * mask[b])                  # [H]

Sharding: data-parallel over B across 8 NeuronCores (512 rows each).
Host prep: the tiny [U,H] user projection + repeat_interleave gather
(~0.05% of FLOPs) and layout marshalling (transposes/casts of inputs,
un-transpose of outputs).

Device pipeline per core, built around two hardware constraints —
matmul PSUM outputs may only start at partition 0/32/64, and per-batch
matvecs are cheapest with the attn tile stationary — so all per-batch
results are produced as PSUM *columns* (free-dim offsets are
unrestricted) and the softmax runs entirely in the transposed
[S, batch] layout:
  - mm1:  stationary W_attr.T [D,H] bf16, moving x.T [D, 4*S] -> z.T [H,512]
  - tanh on ACT with per-partition bias b_attr -> attn_x.T [H, 4*S] bf16
  - score matvec: lhsT = attn_x.T[b] [H,S] (FWL), rhs = ru[b] [H,1]
    -> score.T column -> PSUM [S, BLK]
  - softmax without max-subtraction (scores are O(10), exp is f32-safe):
    mask-mul (DVE) -> exp (ACT) -> den = ones.T @ expT on PE [1,BLK] ->
    reciprocal (DVE) -> broadcast via K=1 ones matmul [S,BLK] ->
    normalize + re-mask (DVE)
  - attn_x.T[b] transposed to natural [S,H] per batch (PE+identity by
    default; xbar DMA-transpose variant behind KVARIANT=dmat)
  - out matvec: lhsT = attn_x[b] [S,H], rhs = (w*mask).T[:,b] [S,1]
    -> out.T column -> PSUM [H, BLK]
Outputs are staged transposed ([H,NB], [S,NB]) and un-transposed on host.
"""

import os

import numpy as np
import ml_dtypes

B, S, D, H, U = 4096, 128, 128, 128, 256
NCORES = 8
NB = B // NCORES          # 512 batches per core
GROUP = 4                 # batches per mm1 matmul (moving N = 512)
XLOAD = 16                # mm1 groups per x DMA (2 MB loads)
BLK = 128                 # batches per softmax block

# "pe":   attn transposes on the tensor engine (no xbar DMA transposes)
# "dmat": attn transposes + x loads via xbar DMA transpose
VARIANT = os.environ.get("KVARIANT", "pe")
F32NORM = os.environ.get("KF32NORM", "1") == "1"  # f32 ones-matmuls for softmax norm

TRACE = False
LAST_RESULTS = None

_compiled = {}


def _build_bass(variant=None):
    import concourse.bacc as bacc
    import concourse.tile as tile
    from concourse import mybir
    from concourse.masks import make_identity

    variant = variant or VARIANT
    f32 = mybir.dt.float32
    bf16 = mybir.dt.float16  # fp16 compute: 10 mantissa bits vs bf16 8
    AF = mybir.ActivationFunctionType
    ALU = mybir.AluOpType
    norm_dt = f32 if F32NORM else bf16

    nc = bacc.Bacc("TRN2", target_bir_lowering=False, debug=False)

    if variant == "dmat":
        xd = nc.dram_tensor("xd", [NB * S, D], bf16, kind="ExternalInput")
    else:
        xd = nc.dram_tensor("xd", [D, NB * S], bf16, kind="ExternalInput")
    masktd = nc.dram_tensor("masktd", [S, NB], f32, kind="ExternalInput")
    rtd = nc.dram_tensor("rtd", [H, NB], bf16, kind="ExternalInput")
    watd = nc.dram_tensor("watd", [D, H], bf16, kind="ExternalInput")
    battrd = nc.dram_tensor("battrd", [H, 1], f32, kind="ExternalInput")
    outxd = nc.dram_tensor("outxd", [H, NB], f32, kind="ExternalOutput")
    outwd = nc.dram_tensor("outwd", [S, NB], f32, kind="ExternalOutput")

    with tile.TileContext(nc) as tc:
        with (
            tc.tile_pool(name="const", bufs=1) as constp,
            tc.tile_pool(name="xin", bufs=3) as xinp,
            tc.tile_pool(name="axt", bufs=6) as axtp,
            tc.tile_pool(name="axn", bufs=(2 * BLK // GROUP)) as axnp,  # [S, ZG*H] tiles
            tc.tile_pool(name="soft", bufs=2) as softp,
            tc.tile_pool(name="zps", bufs=2, space="PSUM") as zpsp,
            tc.tile_pool(name="sps", bufs=1, space="PSUM") as spsp,
            tc.tile_pool(name="nps", bufs=1, space="PSUM") as npsp,
            tc.tile_pool(name="ops", bufs=1, space="PSUM") as opsp,
            tc.tile_pool(name="tps", bufs=2, space="PSUM") as tpsp,
        ):
            wat_sb = constp.tile([D, H], bf16)
            nc.sync.dma_start(wat_sb, watd[:, :])
            battr_sb = constp.tile([H, 1], f32)
            nc.sync.dma_start(battr_sb, battrd[:, :])
            rt_sb = constp.tile([H, NB], bf16)
            nc.sync.dma_start(rt_sb, rtd[:, :])
            maskt_sb = constp.tile([S, NB], f32)
            nc.sync.dma_start(maskt_sb, masktd[:, :])
            ones_dt = f32 if F32NORM else mybir.dt.bfloat16
            ones_col = constp.tile([S, 1], ones_dt)
            nc.vector.memset(ones_col, 1.0)
            ones_row = constp.tile([1, S], ones_dt)
            nc.vector.memset(ones_row, 1.0)
            if variant == "pe":
                ident = constp.tile([128, 128], bf16)
                make_identity(nc, ident)
            outwt_sb = constp.tile([S, NB], f32)
            outxt_sb = constp.tile([H, NB], f32)
            const_dmas = []  # last const DMA gates the first xbar transpose

            for j in range(NB // BLK):
                b0 = j * BLK
                score_ps = spsp.tile([S, BLK], f32, tag="score")
                axn_slices = []
                ZG = 2 * GROUP if variant == "dmat" else GROUP
                for g in range(BLK // ZG):
                    gb = b0 + g * ZG
                    tok0 = gb * S
                    if (g * ZG) % (XLOAD * GROUP) == 0:
                        xt_sb = xinp.tile([D, XLOAD * GROUP * S], bf16, tag="xt")
                        if variant == "dmat":
                            nc.sync.dma_start_transpose(
                                xt_sb, xd[tok0:tok0 + XLOAD * GROUP * S, :])
                        else:
                            nc.sync.dma_start(
                                xt_sb, xd[:, tok0:tok0 + XLOAD * GROUP * S])
                    xoff = (g * ZG * S) % (XLOAD * GROUP * S)
                    z_ps = zpsp.tile([H, ZG * S], f32, tag="z")
                    for h in range(ZG // GROUP):
                        nc.tensor.matmul(
                            z_ps[:, h * GROUP * S:(h + 1) * GROUP * S], wat_sb,
                            xt_sb[:, xoff + h * GROUP * S:xoff + (h + 1) * GROUP * S],
                            start=True, stop=True)
                    axt_sb = axtp.tile([H, ZG * S], bf16, tag="axt")
                    nc.scalar.activation(axt_sb, z_ps, AF.Tanh, bias=battr_sb)

                    if variant == "pe":
                        axn_ps = tpsp.tile([S, ZG * H], bf16, tag="axn_ps")
                    axn_sb = axnp.tile([S, ZG * H], bf16, tag="axn")
                    for i in range(ZG):
                        b = gb + i
                        bi = b - b0
                        a_slice = axt_sb[:, i * S:(i + 1) * S]
                        if variant == "pe":
                            nc.tensor.transpose(
                                axn_ps[:, i * H:(i + 1) * H], a_slice, ident)
                        else:
                            nc.sync.dma_start_transpose(
                                axn_sb[:, i * H:(i + 1) * H], a_slice)
                        nc.tensor.matmul(
                            score_ps[:, bi:bi + 1], a_slice, rt_sb[:, b:b + 1],
                            start=True, stop=True,
                        )
                    if variant == "pe":
                        nc.vector.tensor_copy(axn_sb, axn_ps)
                    for i in range(ZG):
                        axn_slices.append(axn_sb[:, i * H:(i + 1) * H])

                # softmax in transposed layout [S, BLK]
                m_slice = maskt_sb[:, b0:b0 + BLK]
                smt = softp.tile([S, BLK], f32, tag="smt")
                nc.vector.tensor_mul(smt, score_ps, m_slice)
                expt = softp.tile([S, BLK], f32, tag="expt")
                nc.scalar.activation(expt, smt, AF.Exp)
                if F32NORM:
                    expt_n, rden_src = expt, None
                else:
                    # bf16 (not fp16): exp(score) can exceed fp16 max
                    expt_n = softp.tile([S, BLK], mybir.dt.bfloat16, tag="expt_n")
                    nc.vector.tensor_copy(expt_n, expt)
                den_ps = npsp.tile([1, BLK], f32, tag="den")
                nc.tensor.matmul(den_ps, ones_col, expt_n, start=True, stop=True)
                rden = softp.tile([1, BLK], f32, tag="rden")
                nc.vector.reciprocal(rden, den_ps)
                if F32NORM:
                    rden_n = rden
                else:
                    rden_n = softp.tile([1, BLK], mybir.dt.bfloat16, tag="rden_n")
                    nc.vector.tensor_copy(rden_n, rden)
                rb_ps = npsp.tile([S, BLK], f32, tag="rb")
                nc.tensor.matmul(rb_ps, ones_row, rden_n, start=True, stop=True)
                awt_slice = outwt_sb[:, b0:b0 + BLK]
                nc.vector.tensor_mul(awt_slice, expt, rb_ps)
                wmt = softp.tile([S, BLK], bf16, tag="wmt")
                nc.vector.tensor_mul(wmt, awt_slice, m_slice)

                out_ps = opsp.tile([H, BLK], f32, tag="out")
                for bi in range(BLK):
                    nc.tensor.matmul(
                        out_ps[:, bi:bi + 1], axn_slices[bi], wmt[:, bi:bi + 1],
                        start=True, stop=True,
                    )
                nc.vector.tensor_copy(outxt_sb[:, b0:b0 + BLK], out_ps)

            nc.sync.dma_start(outwd[:, :], outwt_sb)
            nc.sync.dma_start(outxd[:, :], outxt_sb)

    nc.compile()
    return nc


def _host_prep(x, x_mask, user, user_size, W_attr, b_attr, W_user, b_user,
               variant=None):
    variant = variant or VARIANT
    x = np.asarray(x)
    x_mask = np.asarray(x_mask, dtype=np.float32)
    user = np.asarray(user, dtype=np.float32)
    user_size = np.asarray(user_size)
    W_attr = np.asarray(W_attr, dtype=np.float32)
    b_attr = np.asarray(b_attr, dtype=np.float32)
    W_user = np.asarray(W_user, dtype=np.float32)
    b_user = np.asarray(b_user, dtype=np.float32)

    # user projection + repeat_interleave gather (tiny; jnp.repeat
    # total_repeat_length semantics: truncate, or pad with final value)
    attn_user = user @ W_user.T + b_user               # [U, H]
    reps = np.repeat(np.arange(U), np.maximum(user_size.astype(np.int64), 0))
    if reps.size >= B:
        idx = reps[:B]
    else:
        pad_val = reps[-1] if reps.size else 0
        idx = np.concatenate([reps, np.full(B - reps.size, pad_val, dtype=np.int64)])
    R = attn_user[idx]                                  # [B, H] f32

    wat_np = np.ascontiguousarray(W_attr.T).astype(np.float16)
    battr_np = np.ascontiguousarray(b_attr.reshape(H, 1))

    in_maps = []
    for c in range(NCORES):
        sl = slice(c * NB, (c + 1) * NB)
        xs = x[sl].astype(np.float16)           # [NB, S, D]
        if variant == "dmat":
            x_np = np.ascontiguousarray(xs.reshape(NB * S, D))
        else:
            x_np = np.ascontiguousarray(xs.reshape(NB * S, D).T)
        maskt_np = np.ascontiguousarray(x_mask[sl].T)
        rt_np = np.ascontiguousarray(R[sl].T.astype(np.float16))
        in_maps.append({
            "xd": x_np, "masktd": maskt_np, "rtd": rt_np,
            "watd": wat_np, "battrd": battr_np,
        })
    return in_maps


def kernel(x, x_mask, user, user_size, W_attr, b_attr, W_user, b_user):
    global LAST_RESULTS
    from concourse.bass_utils import run_bass_kernel_spmd

    in_maps = _host_prep(x, x_mask, user, user_size, W_attr, b_attr,
                         W_user, b_user)
    if VARIANT not in _compiled:
        _compiled[VARIANT] = _build_bass()
    LAST_RESULTS = run_bass_kernel_spmd(
        _compiled[VARIANT], in_maps, core_ids=list(range(NCORES)), trace=TRACE,
    )
    outs = LAST_RESULTS.results
    attn_weighted_x = np.concatenate(
        [np.ascontiguousarray(o["outxd"].T) for o in outs], axis=0)
    attn_weight = np.concatenate(
        [np.ascontiguousarray(o["outwd"].T) for o in outs], axis=0)
    return attn_weighted_x, attn_weight


# revision 14
# speedup vs baseline: 5.5288x; 5.5288x over previous
"""Trainium2 Bass kernel for the ATTR_TRANSFORMER pooling module.

Computation (per batch row b):
    attn_x  = tanh(x[b] @ W_attr.T + b_attr)            # [S, H]
    ru      = (user @ W_user.T + b_user)[idx[b]]        # [H]
    score   = (attn_x * mask[b,:,None]) @ ru            # [S]
    w       = softmax(score)                            # [S]
    out_x   = attn_x.T @ (w * mask[b])                  # [H]

Sharding: data-parallel over B across 8 NeuronCores (512 rows each).
Host prep: the tiny [U,H] user projection + repeat_interleave gather
(~0.05% of FLOPs) and layout marshalling (transposes/casts of inputs,
un-transpose of outputs).

Device pipeline per core, built around two hardware constraints —
matmul PSUM outputs may only start at partition 0/32/64, and per-batch
matvecs are cheapest with the attn tile stationary — so all per-batch
results are produced as PSUM *columns* (free-dim offsets are
unrestricted) and the softmax runs entirely in the transposed
[S, batch] layout:
  - mm1:  stationary W_attr.T [D,H] bf16, moving x.T [D, 4*S] -> z.T [H,512]
  - tanh on ACT with per-partition bias b_attr -> attn_x.T [H, 4*S] bf16
  - score matvec: lhsT = attn_x.T[b] [H,S] (FWL), rhs = ru[b] [H,1]
    -> score.T column -> PSUM [S, BLK]
  - softmax without max-subtraction (scores are O(10), exp is f32-safe):
    mask-mul (DVE) -> exp (ACT) -> den = ones.T @ expT on PE [1,BLK] ->
    reciprocal (DVE) -> broadcast via K=1 ones matmul [S,BLK] ->
    normalize + re-mask (DVE)
  - attn_x.T[b] transposed to natural [S,H] per batch (PE+identity by
    default; xbar DMA-transpose variant behind KVARIANT=dmat)
  - out matvec: lhsT = attn_x[b] [S,H], rhs = (w*mask).T[:,b] [S,1]
    -> out.T column -> PSUM [H, BLK]
Outputs are staged transposed ([H,NB], [S,NB]) and un-transposed on host.
"""

import os

import numpy as np
import ml_dtypes

B, S, D, H, U = 4096, 128, 128, 128, 256
NCORES = 8
NB = B // NCORES          # 512 batches per core
GROUP = 4                 # batches per mm1 matmul (moving N = 512)
XLOAD = 4                 # mm1 groups per x DMA (512 KB loads)
BLK = 128                 # batches per softmax block

# "pe":   attn transposes on the tensor engine (no xbar DMA transposes)
# "dmat": attn transposes + x loads via xbar DMA transpose
VARIANT = os.environ.get("KVARIANT", "pe")
F32NORM = os.environ.get("KF32NORM", "1") == "1"  # f32 ones-matmuls for softmax norm

TRACE = False
LAST_RESULTS = None

_compiled = {}


def _build_bass(variant=None):
    import concourse.bacc as bacc
    import concourse.tile as tile
    from concourse import mybir
    from concourse.masks import make_identity

    variant = variant or VARIANT
    f32 = mybir.dt.float32
    bf16 = mybir.dt.float16  # fp16 compute: 10 mantissa bits vs bf16 8
    AF = mybir.ActivationFunctionType
    ALU = mybir.AluOpType
    norm_dt = f32 if F32NORM else bf16

    nc = bacc.Bacc("TRN2", target_bir_lowering=False, debug=False)

    if variant == "dmat":
        xd = nc.dram_tensor("xd", [NB * S, D], bf16, kind="ExternalInput")
    else:
        xd = nc.dram_tensor("xd", [D, NB * S], bf16, kind="ExternalInput")
    masktd = nc.dram_tensor("masktd", [S, NB], f32, kind="ExternalInput")
    rtd = nc.dram_tensor("rtd", [H, NB], bf16, kind="ExternalInput")
    watd = nc.dram_tensor("watd", [D, H], bf16, kind="ExternalInput")
    battrd = nc.dram_tensor("battrd", [H, 1], f32, kind="ExternalInput")
    outxd = nc.dram_tensor("outxd", [H, NB], f32, kind="ExternalOutput")
    outwd = nc.dram_tensor("outwd", [S, NB], f32, kind="ExternalOutput")

    with tile.TileContext(nc) as tc:
        with (
            tc.tile_pool(name="const", bufs=1) as constp,
            tc.tile_pool(name="xin", bufs=6) as xinp,
            tc.tile_pool(name="axt", bufs=6) as axtp,
            tc.tile_pool(name="axn", bufs=(BLK // GROUP)) as axnp,  # [S, ZG*H] tiles x 2 blocks
            tc.tile_pool(name="soft", bufs=2) as softp,
            tc.tile_pool(name="zps", bufs=2, space="PSUM") as zpsp,
            tc.tile_pool(name="sps", bufs=1, space="PSUM") as spsp,
            tc.tile_pool(name="nps", bufs=1, space="PSUM") as npsp,
            tc.tile_pool(name="ops", bufs=1, space="PSUM") as opsp,
            tc.tile_pool(name="tps", bufs=1, space="PSUM") as tpsp,
        ):
            wat_sb = constp.tile([D, H], bf16)
            nc.sync.dma_start(wat_sb, watd[:, :])
            battr_sb = constp.tile([H, 1], f32)
            nc.sync.dma_start(battr_sb, battrd[:, :])
            rt_sb = constp.tile([H, NB], bf16)
            nc.sync.dma_start(rt_sb, rtd[:, :])
            # prefetch block 0's first x tile before the bulky mask DMA so the
            # PE pipeline ramps immediately
            xpre_sb = constp.tile([D, XLOAD * GROUP * S], bf16, name="xpre")
            if variant == "dmat":
                nc.sync.dma_start_transpose(xpre_sb, xd[0:XLOAD * GROUP * S, :])
            else:
                nc.sync.dma_start(xpre_sb, xd[:, 0:XLOAD * GROUP * S])
            maskt_sb = constp.tile([S, NB], f32)
            nc.sync.dma_start(maskt_sb, masktd[:, :])
            ones_dt = f32 if F32NORM else mybir.dt.bfloat16
            ones_col = constp.tile([S, 1], ones_dt)
            nc.vector.memset(ones_col, 1.0)
            ones_row = constp.tile([1, S], ones_dt)
            nc.vector.memset(ones_row, 1.0)
            if variant == "pe":
                ident = constp.tile([128, 128], bf16)
                make_identity(nc, ident)
            outwt_sb = constp.tile([S, NB], f32)
            outxt_sb = constp.tile([H, NB], f32)
            const_dmas = []  # last const DMA gates the first xbar transpose

            def emit_front(j):
                """mm1 + tanh + score matvecs + transposes + softmax for block j.
                Returns (axn_slices, wmt) needed by the deferred out matvecs."""
                b0 = j * BLK
                score_ps = spsp.tile([S, BLK], f32, tag="score", name=f"score{j}")
                axn_slices = []
                ZG = 2 * GROUP
                for g in range(BLK // ZG):
                    gb = b0 + g * ZG
                    tok0 = gb * S
                    if (g * ZG) % (XLOAD * GROUP) == 0:
                        if j == 0 and g == 0:
                            emit_front.xt_sb = xpre_sb
                        else:
                            xt_sb = xinp.tile([D, XLOAD * GROUP * S], bf16,
                                              tag="xt", name=f"xt{j}_{g}")
                            if variant == "dmat":
                                nc.sync.dma_start_transpose(
                                    xt_sb, xd[tok0:tok0 + XLOAD * GROUP * S, :])
                            else:
                                nc.sync.dma_start(
                                    xt_sb, xd[:, tok0:tok0 + XLOAD * GROUP * S])
                            emit_front.xt_sb = xt_sb
                    xt_sb = emit_front.xt_sb
                    xoff = (g * ZG * S) % (XLOAD * GROUP * S)
                    z_ps = zpsp.tile([H, ZG * S], f32, tag="z", name=f"z{j}_{g}")
                    for h in range(ZG // GROUP):
                        nc.tensor.matmul(
                            z_ps[:, h * GROUP * S:(h + 1) * GROUP * S], wat_sb,
                            xt_sb[:, xoff + h * GROUP * S:xoff + (h + 1) * GROUP * S],
                            start=True, stop=True)
                    axt_sb = axtp.tile([H, ZG * S], bf16, tag="axt",
                                       name=f"axt{j}_{g}")
                    nc.scalar.activation(axt_sb, z_ps, AF.Tanh, bias=battr_sb)

                    if variant == "pe":
                        axn_ps = tpsp.tile([S, ZG * H], bf16, tag="axn_ps",
                                           name=f"axn_ps{j}_{g}")
                    axn_sb = axnp.tile([S, ZG * H], bf16, tag="axn",
                                       name=f"axn{j}_{g}")
                    for i in range(ZG):
                        b = gb + i
                        bi = b - b0
                        a_slice = axt_sb[:, i * S:(i + 1) * S]
                        nc.tensor.matmul(
                            score_ps[:, bi:bi + 1], a_slice, rt_sb[:, b:b + 1],
                            start=True, stop=True,
                        )
                        if variant == "pe":
                            nc.tensor.transpose(
                                axn_ps[:, i * H:(i + 1) * H], a_slice, ident)
                        else:
                            nc.sync.dma_start_transpose(
                                axn_sb[:, i * H:(i + 1) * H], a_slice)
                    if variant == "pe":
                        nc.vector.tensor_copy(axn_sb, axn_ps)
                    for i in range(ZG):
                        axn_slices.append(axn_sb[:, i * H:(i + 1) * H])

                # softmax in transposed layout [S, BLK]
                m_slice = maskt_sb[:, b0:b0 + BLK]
                smt = softp.tile([S, BLK], f32, tag="smt", name=f"smt{j}")
                nc.vector.tensor_mul(smt, score_ps, m_slice)
                expt = softp.tile([S, BLK], f32, tag="expt", name=f"expt{j}")
                nc.scalar.activation(expt, smt, AF.Exp)
                if F32NORM:
                    expt_n = expt
                else:
                    expt_n = softp.tile([S, BLK], mybir.dt.bfloat16,
                                        tag="expt_n", name=f"expt_n{j}")
                    nc.vector.tensor_copy(expt_n, expt)
                den_ps = npsp.tile([1, BLK], f32, tag="nrm", name=f"den_ps{j}")
                nc.tensor.matmul(den_ps, ones_col, expt_n, start=True, stop=True)
                rden = softp.tile([1, BLK], f32, tag="rden", name=f"rden{j}")
                nc.vector.reciprocal(rden, den_ps)
                if F32NORM:
                    rden_n = rden
                else:
                    rden_n = softp.tile([1, BLK], mybir.dt.bfloat16,
                                        tag="rden_n", name=f"rden_n{j}")
                    nc.vector.tensor_copy(rden_n, rden)
                rb_ps = npsp.tile([S, BLK], f32, tag="nrm", name=f"rb_ps{j}")
                nc.tensor.matmul(rb_ps, ones_row, rden_n, start=True, stop=True)
                awt_slice = outwt_sb[:, b0:b0 + BLK]
                nc.vector.tensor_mul(awt_slice, expt, rb_ps)
                wmt = softp.tile([S, BLK], bf16, tag="wmt", name=f"wmt{j}")
                nc.vector.tensor_mul(wmt, awt_slice, m_slice)
                return axn_slices, wmt

            def emit_out(j, axn_slices, wmt):
                b0 = j * BLK
                out_ps = opsp.tile([H, BLK], f32, tag="out", name=f"out{j}")
                for bi in range(BLK):
                    nc.tensor.matmul(
                        out_ps[:, bi:bi + 1], axn_slices[bi], wmt[:, bi:bi + 1],
                        start=True, stop=True,
                    )
                nc.vector.tensor_copy(outxt_sb[:, b0:b0 + BLK], out_ps)

            # software pipeline: block j's out matvecs are emitted after
            # block j+1's scores, so the PE never waits on a softmax chain
            pending = None
            for j in range(NB // BLK):
                front = emit_front(j)
                if pending is not None:
                    emit_out(pending[0], pending[1], pending[2])
                pending = (j, front[0], front[1])
            emit_out(pending[0], pending[1], pending[2])

            nc.sync.dma_start(outwd[:, :], outwt_sb)
            nc.sync.dma_start(outxd[:, :], outxt_sb)

    nc.compile()
    return nc


def _host_prep(x, x_mask, user, user_size, W_attr, b_attr, W_user, b_user,
               variant=None):
    variant = variant or VARIANT
    x = np.asarray(x)
    x_mask = np.asarray(x_mask, dtype=np.float32)
    user = np.asarray(user, dtype=np.float32)
    user_size = np.asarray(user_size)
    W_attr = np.asarray(W_attr, dtype=np.float32)
    b_attr = np.asarray(b_attr, dtype=np.float32)
    W_user = np.asarray(W_user, dtype=np.float32)
    b_user = np.asarray(b_user, dtype=np.float32)

    # user projection + repeat_interleave gather (tiny; jnp.repeat
    # total_repeat_length semantics: truncate, or pad with final value)
    attn_user = user @ W_user.T + b_user               # [U, H]
    reps = np.repeat(np.arange(U), np.maximum(user_size.astype(np.int64), 0))
    if reps.size >= B:
        idx = reps[:B]
    else:
        pad_val = reps[-1] if reps.size else 0
        idx = np.concatenate([reps, np.full(B - reps.size, pad_val, dtype=np.int64)])
    R = attn_user[idx]                                  # [B, H] f32

    wat_np = np.ascontiguousarray(W_attr.T).astype(np.float16)
    battr_np = np.ascontiguousarray(b_attr.reshape(H, 1))

    in_maps = []
    for c in range(NCORES):
        sl = slice(c * NB, (c + 1) * NB)
        xs = x[sl].astype(np.float16)           # [NB, S, D]
        if variant == "dmat":
            x_np = np.ascontiguousarray(xs.reshape(NB * S, D))
        else:
            x_np = np.ascontiguousarray(xs.reshape(NB * S, D).T)
        maskt_np = np.ascontiguousarray(x_mask[sl].T)
        rt_np = np.ascontiguousarray(R[sl].T.astype(np.float16))
        in_maps.append({
            "xd": x_np, "masktd": maskt_np, "rtd": rt_np,
            "watd": wat_np, "battrd": battr_np,
        })
    return in_maps


def kernel(x, x_mask, user, user_size, W_attr, b_attr, W_user, b_user):
    global LAST_RESULTS
    from concourse.bass_utils import run_bass_kernel_spmd

    in_maps = _host_prep(x, x_mask, user, user_size, W_attr, b_attr,
                         W_user, b_user)
    if VARIANT not in _compiled:
        _compiled[VARIANT] = _build_bass()
    LAST_RESULTS = run_bass_kernel_spmd(
        _compiled[VARIANT], in_maps, core_ids=list(range(NCORES)), trace=TRACE,
    )
    outs = LAST_RESULTS.results
    attn_weighted_x = np.concatenate(
        [np.ascontiguousarray(o["outxd"].T) for o in outs], axis=0)
    attn_weight = np.concatenate(
        [np.ascontiguousarray(o["outwd"].T) for o in outs], axis=0)
    return attn_weighted_x, attn_weight


# revision 17
# speedup vs baseline: 5.5382x; 1.0017x over previous
"""Trainium2 Bass kernel for the ATTR_TRANSFORMER pooling module.

Computation (per batch row b):
    attn_x  = tanh(x[b] @ W_attr.T + b_attr)            # [S, H]
    ru      = (user @ W_user.T + b_user)[idx[b]]        # [H]
    score   = (attn_x * mask[b,:,None]) @ ru            # [S]
    w       = softmax(score)                            # [S]
    out_x   = attn_x.T @ (w * mask[b])                  # [H]

Sharding: data-parallel over B across 8 NeuronCores (512 rows each).
Host prep: the tiny [U,H] user projection + repeat_interleave gather
(~0.05% of FLOPs) and layout marshalling (transposes/casts of inputs,
un-transpose of outputs).

Device pipeline per core, built around two hardware constraints —
matmul PSUM outputs may only start at partition 0/32/64, and per-batch
matvecs are cheapest with the attn tile stationary — so all per-batch
results are produced as PSUM *columns* (free-dim offsets are
unrestricted) and the softmax runs entirely in the transposed
[S, batch] layout:
  - mm1:  stationary W_attr.T [D,H] bf16, moving x.T [D, 4*S] -> z.T [H,512]
  - tanh on ACT with per-partition bias b_attr -> attn_x.T [H, 4*S] bf16
  - score matvec: lhsT = attn_x.T[b] [H,S] (FWL), rhs = ru[b] [H,1]
    -> score.T column -> PSUM [S, BLK]
  - softmax without max-subtraction (scores are O(10), exp is f32-safe):
    mask-mul (DVE) -> exp (ACT) -> den = ones.T @ expT on PE [1,BLK] ->
    reciprocal (DVE) -> broadcast via K=1 ones matmul [S,BLK] ->
    normalize + re-mask (DVE)
  - attn_x.T[b] transposed to natural [S,H] per batch (PE+identity by
    default; xbar DMA-transpose variant behind KVARIANT=dmat)
  - out matvec: lhsT = attn_x[b] [S,H], rhs = (w*mask).T[:,b] [S,1]
    -> out.T column -> PSUM [H, BLK]
Outputs are staged transposed ([H,NB], [S,NB]) and un-transposed on host.
"""

import os

import numpy as np


B, S, D, H, U = 4096, 128, 128, 128, 256
NCORES = 8
NB = B // NCORES          # 512 batches per core
GROUP = 4                 # batches per mm1 matmul (moving N = 512)
XLOAD = 4                 # mm1 groups per x DMA (512 KB loads)
BLK = 128                 # batches per softmax block

# "pe":   attn transposes on the tensor engine (no xbar DMA transposes)
# "dmat": attn transposes + x loads via xbar DMA transpose
VARIANT = os.environ.get("KVARIANT", "pe")
F32NORM = os.environ.get("KF32NORM", "1") == "1"  # f32 ones-matmuls for softmax norm

TRACE = False
LAST_RESULTS = None

_compiled = {}


def _build_bass(variant=None):
    import concourse.bacc as bacc
    import concourse.tile as tile
    from concourse import mybir
    from concourse.masks import make_identity

    variant = variant or VARIANT
    f32 = mybir.dt.float32
    bf16 = mybir.dt.float16  # fp16 compute: 10 mantissa bits vs bf16 8
    AF = mybir.ActivationFunctionType
    ALU = mybir.AluOpType
    norm_dt = f32 if F32NORM else bf16

    nc = bacc.Bacc("TRN2", target_bir_lowering=False, debug=False)

    if variant == "dmat":
        xd = nc.dram_tensor("xd", [NB * S, D], bf16, kind="ExternalInput")
    else:
        xd = nc.dram_tensor("xd", [D, NB * S], bf16, kind="ExternalInput")
    masktd = nc.dram_tensor("masktd", [S, NB], f32, kind="ExternalInput")
    rtd = nc.dram_tensor("rtd", [H, NB], bf16, kind="ExternalInput")
    watd = nc.dram_tensor("watd", [D, H], bf16, kind="ExternalInput")
    battrd = nc.dram_tensor("battrd", [H, 1], f32, kind="ExternalInput")
    outxd = nc.dram_tensor("outxd", [H, NB], f32, kind="ExternalOutput")
    outwd = nc.dram_tensor("outwd", [S, NB], f32, kind="ExternalOutput")

    with tile.TileContext(nc) as tc:
        with (
            tc.tile_pool(name="const", bufs=1) as constp,
            tc.tile_pool(name="xin", bufs=6) as xinp,
            tc.tile_pool(name="axt", bufs=6) as axtp,
            tc.tile_pool(name="axn", bufs=(BLK // GROUP)) as axnp,  # [S, ZG*H] tiles x 2 blocks
            tc.tile_pool(name="soft", bufs=2) as softp,
            tc.tile_pool(name="zps", bufs=2, space="PSUM") as zpsp,
            tc.tile_pool(name="sps", bufs=1, space="PSUM") as spsp,
            tc.tile_pool(name="nps", bufs=1, space="PSUM") as npsp,
            tc.tile_pool(name="ops", bufs=1, space="PSUM") as opsp,
            tc.tile_pool(name="tps", bufs=1, space="PSUM") as tpsp,
        ):
            wat_sb = constp.tile([D, H], bf16)
            nc.sync.dma_start(wat_sb, watd[:, :])
            battr_sb = constp.tile([H, 1], f32)
            nc.sync.dma_start(battr_sb, battrd[:, :])
            rt_sb = constp.tile([H, NB], bf16)
            nc.sync.dma_start(rt_sb, rtd[:, :])
            # prefetch block 0's first x tile before the bulky mask DMA so the
            # PE pipeline ramps immediately
            xpre_sb = constp.tile([D, XLOAD * GROUP * S], bf16, name="xpre")
            if variant == "dmat":
                nc.sync.dma_start_transpose(xpre_sb, xd[0:XLOAD * GROUP * S, :])
            else:
                nc.sync.dma_start(xpre_sb, xd[:, 0:XLOAD * GROUP * S])
            maskt_sb = constp.tile([S, NB], f32)
            nc.sync.dma_start(maskt_sb, masktd[:, :])
            ones_dt = f32 if F32NORM else mybir.dt.bfloat16
            ones_col = constp.tile([S, 1], ones_dt)
            nc.vector.memset(ones_col, 1.0)
            ones_row = constp.tile([1, S], ones_dt)
            nc.vector.memset(ones_row, 1.0)
            if variant == "pe":
                ident = constp.tile([128, 128], bf16)
                make_identity(nc, ident)
            outwt_sb = constp.tile([S, NB], f32)
            outxt_sb = constp.tile([H, NB], f32)
            const_dmas = []  # last const DMA gates the first xbar transpose

            def emit_front(j):
                """mm1 + tanh + score matvecs + transposes + softmax for block j.
                Returns (axn_slices, wmt) needed by the deferred out matvecs."""
                b0 = j * BLK
                score_ps = spsp.tile([S, BLK], f32, tag="score", name=f"score{j}")
                axn_slices = []
                ZG = 2 * GROUP
                for g in range(BLK // ZG):
                    gb = b0 + g * ZG
                    tok0 = gb * S
                    if (g * ZG) % (XLOAD * GROUP) == 0:
                        if j == 0 and g == 0:
                            emit_front.xt_sb = xpre_sb
                        else:
                            xt_sb = xinp.tile([D, XLOAD * GROUP * S], bf16,
                                              tag="xt", name=f"xt{j}_{g}")
                            if variant == "dmat":
                                nc.sync.dma_start_transpose(
                                    xt_sb, xd[tok0:tok0 + XLOAD * GROUP * S, :])
                            else:
                                nc.sync.dma_start(
                                    xt_sb, xd[:, tok0:tok0 + XLOAD * GROUP * S])
                            emit_front.xt_sb = xt_sb
                    xt_sb = emit_front.xt_sb
                    xoff = (g * ZG * S) % (XLOAD * GROUP * S)
                    z_ps = zpsp.tile([H, ZG * S], f32, tag="z", name=f"z{j}_{g}")
                    for h in range(ZG // GROUP):
                        nc.tensor.matmul(
                            z_ps[:, h * GROUP * S:(h + 1) * GROUP * S], wat_sb,
                            xt_sb[:, xoff + h * GROUP * S:xoff + (h + 1) * GROUP * S],
                            start=True, stop=True)
                    axt_sb = axtp.tile([H, ZG * S], bf16, tag="axt",
                                       name=f"axt{j}_{g}")
                    nc.scalar.activation(axt_sb, z_ps, AF.Tanh, bias=battr_sb)

                    if variant == "pe":
                        axn_ps = tpsp.tile([S, ZG * H], bf16, tag="axn_ps",
                                           name=f"axn_ps{j}_{g}")
                    axn_sb = axnp.tile([S, ZG * H], bf16, tag="axn",
                                       name=f"axn{j}_{g}")
                    for i in range(ZG):
                        b = gb + i
                        bi = b - b0
                        a_slice = axt_sb[:, i * S:(i + 1) * S]
                        nc.tensor.matmul(
                            score_ps[:, bi:bi + 1], a_slice, rt_sb[:, b:b + 1],
                            start=True, stop=True,
                        )
                        if variant == "pe":
                            nc.tensor.transpose(
                                axn_ps[:, i * H:(i + 1) * H], a_slice, ident)
                        else:
                            nc.sync.dma_start_transpose(
                                axn_sb[:, i * H:(i + 1) * H], a_slice)
                    if variant == "pe":
                        nc.vector.tensor_copy(axn_sb, axn_ps)
                    for i in range(ZG):
                        axn_slices.append(axn_sb[:, i * H:(i + 1) * H])

                # softmax in transposed layout [S, BLK]
                m_slice = maskt_sb[:, b0:b0 + BLK]
                smt = softp.tile([S, BLK], f32, tag="smt", name=f"smt{j}")
                nc.vector.tensor_mul(smt, score_ps, m_slice)
                expt = softp.tile([S, BLK], f32, tag="expt", name=f"expt{j}")
                nc.scalar.activation(expt, smt, AF.Exp)
                if F32NORM:
                    expt_n = expt
                else:
                    expt_n = softp.tile([S, BLK], mybir.dt.bfloat16,
                                        tag="expt_n", name=f"expt_n{j}")
                    nc.vector.tensor_copy(expt_n, expt)
                den_ps = npsp.tile([1, BLK], f32, tag="nrm", name=f"den_ps{j}")
                nc.tensor.matmul(den_ps, ones_col, expt_n, start=True, stop=True)
                rden = softp.tile([1, BLK], f32, tag="rden", name=f"rden{j}")
                nc.vector.reciprocal(rden, den_ps)
                if F32NORM:
                    rden_n = rden
                else:
                    rden_n = softp.tile([1, BLK], mybir.dt.bfloat16,
                                        tag="rden_n", name=f"rden_n{j}")
                    nc.vector.tensor_copy(rden_n, rden)
                rb_ps = npsp.tile([S, BLK], f32, tag="nrm", name=f"rb_ps{j}")
                nc.tensor.matmul(rb_ps, ones_row, rden_n, start=True, stop=True)
                awt_slice = outwt_sb[:, b0:b0 + BLK]
                nc.vector.tensor_mul(awt_slice, expt, rb_ps)
                wmt = softp.tile([S, BLK], bf16, tag="wmt", name=f"wmt{j}")
                nc.vector.tensor_mul(wmt, awt_slice, m_slice)
                return axn_slices, wmt

            def emit_out(j, axn_slices, wmt):
                b0 = j * BLK
                out_ps = opsp.tile([H, BLK], f32, tag="out", name=f"out{j}")
                for bi in range(BLK):
                    nc.tensor.matmul(
                        out_ps[:, bi:bi + 1], axn_slices[bi], wmt[:, bi:bi + 1],
                        start=True, stop=True,
                    )
                nc.vector.tensor_copy(outxt_sb[:, b0:b0 + BLK], out_ps)

            # software pipeline: block j's out matvecs are emitted after
            # block j+1's scores, so the PE never waits on a softmax chain
            pending = None
            for j in range(NB // BLK):
                front = emit_front(j)
                if pending is not None:
                    emit_out(pending[0], pending[1], pending[2])
                pending = (j, front[0], front[1])
            emit_out(pending[0], pending[1], pending[2])

            nc.sync.dma_start(outwd[:, :], outwt_sb)
            nc.sync.dma_start(outxd[:, :], outxt_sb)

    nc.compile()
    return nc


def _host_prep(x, x_mask, user, user_size, W_attr, b_attr, W_user, b_user,
               variant=None):
    variant = variant or VARIANT
    x = np.asarray(x)
    x_mask = np.asarray(x_mask, dtype=np.float32)
    user = np.asarray(user, dtype=np.float32)
    user_size = np.asarray(user_size)
    W_attr = np.asarray(W_attr, dtype=np.float32)
    b_attr = np.asarray(b_attr, dtype=np.float32)
    W_user = np.asarray(W_user, dtype=np.float32)
    b_user = np.asarray(b_user, dtype=np.float32)

    # user projection + repeat_interleave gather (tiny; jnp.repeat
    # total_repeat_length semantics: truncate, or pad with final value)
    attn_user = user @ W_user.T + b_user               # [U, H]
    reps = np.repeat(np.arange(U), np.maximum(user_size.astype(np.int64), 0))
    if reps.size >= B:
        idx = reps[:B]
    else:
        pad_val = reps[-1] if reps.size else 0
        idx = np.concatenate([reps, np.full(B - reps.size, pad_val, dtype=np.int64)])
    R = attn_user[idx]                                  # [B, H] f32

    wat_np = np.ascontiguousarray(W_attr.T).astype(np.float16)
    battr_np = np.ascontiguousarray(b_attr.reshape(H, 1))

    in_maps = []
    for c in range(NCORES):
        sl = slice(c * NB, (c + 1) * NB)
        xs = x[sl].astype(np.float16)           # [NB, S, D]
        if variant == "dmat":
            x_np = np.ascontiguousarray(xs.reshape(NB * S, D))
        else:
            x_np = np.ascontiguousarray(xs.reshape(NB * S, D).T)
        maskt_np = np.ascontiguousarray(x_mask[sl].T)
        rt_np = np.ascontiguousarray(R[sl].T.astype(np.float16))
        in_maps.append({
            "xd": x_np, "masktd": maskt_np, "rtd": rt_np,
            "watd": wat_np, "battrd": battr_np,
        })
    return in_maps


def kernel(x, x_mask, user, user_size, W_attr, b_attr, W_user, b_user):
    global LAST_RESULTS
    from concourse.bass_utils import run_bass_kernel_spmd

    in_maps = _host_prep(x, x_mask, user, user_size, W_attr, b_attr,
                         W_user, b_user)
    if VARIANT not in _compiled:
        _compiled[VARIANT] = _build_bass()
    LAST_RESULTS = run_bass_kernel_spmd(
        _compiled[VARIANT], in_maps, core_ids=list(range(NCORES)), trace=TRACE,
    )
    outs = LAST_RESULTS.results
    attn_weighted_x = np.concatenate(
        [np.ascontiguousarray(o["outxd"].T) for o in outs], axis=0)
    attn_weight = np.concatenate(
        [np.ascontiguousarray(o["outwd"].T) for o in outs], axis=0)
    return attn_weighted_x, attn_weight


# revision 21
# speedup vs baseline: 5.6995x; 1.0291x over previous
"""Trainium2 Bass kernel for the ATTR_TRANSFORMER pooling module.

Computation (per batch row b):
    attn_x  = tanh(x[b] @ W_attr.T + b_attr)            # [S, H]
    ru      = (user @ W_user.T + b_user)[idx[b]]        # [H]
    score   = (attn_x * mask[b,:,None]) @ ru            # [S]
    w       = softmax(score)                            # [S]
    out_x   = attn_x.T @ (w * mask[b])                  # [H]

Sharding: data-parallel over B across 8 NeuronCores (512 rows each).
Host prep: the tiny [U,H] user projection + repeat_interleave gather
(~0.05% of FLOPs) and layout marshalling (transposes/casts of inputs,
un-transpose of outputs).

Device pipeline per core, built around two hardware constraints —
matmul PSUM outputs may only start at partition 0/32/64, and per-batch
matvecs are cheapest with the attn tile stationary — so all per-batch
results are produced as PSUM *columns* (free-dim offsets are
unrestricted) and the softmax runs entirely in the transposed
[S, batch] layout:
  - mm1:  stationary W_attr.T [D,H] bf16, moving x.T [D, 4*S] -> z.T [H,512]
  - tanh on ACT with per-partition bias b_attr -> attn_x.T [H, 4*S] bf16
  - score matvec: lhsT = attn_x.T[b] [H,S] (FWL), rhs = ru[b] [H,1]
    -> score.T column -> PSUM [S, BLK]
  - softmax without max-subtraction (scores are O(10), exp is f32-safe):
    mask-mul (DVE) -> exp (ACT) -> den = ones.T @ expT on PE [1,BLK] ->
    reciprocal (DVE) -> broadcast via K=1 ones matmul [S,BLK] ->
    normalize + re-mask (DVE)
  - attn_x.T[b] transposed to natural [S,H] per batch (PE+identity by
    default; xbar DMA-transpose variant behind KVARIANT=dmat)
  - out matvec: lhsT = attn_x[b] [S,H], rhs = (w*mask).T[:,b] [S,1]
    -> out.T column -> PSUM [H, BLK]
Outputs are staged transposed ([H,NB], [S,NB]) and un-transposed on host.
"""

import os

import numpy as np


B, S, D, H, U = 4096, 128, 128, 128, 256
NCORES = 8
NB = B // NCORES          # 512 batches per core
GROUP = 4                 # batches per mm1 matmul (moving N = 512)
XLOAD = 4                 # mm1 groups per x DMA (512 KB loads)
BLK = 128                 # batches per softmax block

# "pe":   attn transposes on the tensor engine (no xbar DMA transposes)
# "dmat": attn transposes + x loads via xbar DMA transpose
VARIANT = os.environ.get("KVARIANT", "pe")
F32NORM = os.environ.get("KF32NORM", "1") == "1"  # f32 ones-matmuls for softmax norm

TRACE = False
LAST_RESULTS = None

_compiled = {}


def _build_bass(variant=None):
    import concourse.bacc as bacc
    import concourse.tile as tile
    from concourse import mybir
    from concourse.masks import make_identity

    variant = variant or VARIANT
    f32 = mybir.dt.float32
    bf16 = mybir.dt.float16  # fp16 compute: 10 mantissa bits vs bf16 8
    AF = mybir.ActivationFunctionType
    ALU = mybir.AluOpType
    norm_dt = f32 if F32NORM else bf16

    nc = bacc.Bacc("TRN2", target_bir_lowering=False, debug=False)

    if variant == "dmat":
        xd = nc.dram_tensor("xd", [NB * S, D], bf16, kind="ExternalInput")
    else:
        xd = nc.dram_tensor("xd", [D, NB * S], bf16, kind="ExternalInput")
    masktd = nc.dram_tensor("masktd", [S, NB], f32, kind="ExternalInput")
    rtd = nc.dram_tensor("rtd", [H, NB], bf16, kind="ExternalInput")
    watd = nc.dram_tensor("watd", [D, H], bf16, kind="ExternalInput")
    battrd = nc.dram_tensor("battrd", [H, 1], f32, kind="ExternalInput")
    outxd = nc.dram_tensor("outxd", [H, NB], f32, kind="ExternalOutput")
    outwd = nc.dram_tensor("outwd", [S, NB], f32, kind="ExternalOutput")

    with tile.TileContext(nc) as tc:
        with (
            tc.tile_pool(name="const", bufs=1) as constp,
            tc.tile_pool(name="xin", bufs=6) as xinp,
            tc.tile_pool(name="axt", bufs=8) as axtp,
            tc.tile_pool(name="axn", bufs=(BLK // GROUP)) as axnp,  # [S, ZG*H] tiles x 2 blocks
            tc.tile_pool(name="soft", bufs=3) as softp,
            tc.tile_pool(name="zps", bufs=2, space="PSUM") as zpsp,
            tc.tile_pool(name="sps", bufs=1, space="PSUM") as spsp,
            tc.tile_pool(name="nps", bufs=1, space="PSUM") as npsp,
            tc.tile_pool(name="ops", bufs=1, space="PSUM") as opsp,
            tc.tile_pool(name="tps", bufs=1, space="PSUM") as tpsp,
        ):
            wat_sb = constp.tile([D, H], bf16)
            nc.gpsimd.dma_start(wat_sb, watd[:, :])
            battr_sb = constp.tile([H, 1], f32)
            nc.gpsimd.dma_start(battr_sb, battrd[:, :])
            rt_sb = constp.tile([H, NB], bf16)
            nc.gpsimd.dma_start(rt_sb, rtd[:, :])
            # prefetch block 0's first x tile before the bulky mask DMA so the
            # PE pipeline ramps immediately
            xpre_sb = constp.tile([D, XLOAD * GROUP * S], bf16, name="xpre")
            if variant == "dmat":
                nc.sync.dma_start_transpose(xpre_sb, xd[0:XLOAD * GROUP * S, :])
            else:
                nc.sync.dma_start(xpre_sb, xd[:, 0:XLOAD * GROUP * S])
            maskt_sb = constp.tile([S, NB], f32)
            nc.gpsimd.dma_start(maskt_sb, masktd[:, :])
            warm = constp.tile([1, 1], f32)
            nc.vector.memset(warm, 0.0)
            nc.scalar.activation(warm, warm, AF.Tanh)  # prefetch ACT table set
            ones_dt = f32 if F32NORM else mybir.dt.bfloat16
            ones_col = constp.tile([S, 1], ones_dt)
            nc.vector.memset(ones_col, 1.0)
            ones_row = constp.tile([1, S], ones_dt)
            nc.vector.memset(ones_row, 1.0)
            if variant == "pe":
                ident = constp.tile([128, 128], bf16)
                make_identity(nc, ident)
            outwt_sb = constp.tile([S, NB], f32)
            outxt_sb = constp.tile([H, NB], f32)
            const_dmas = []  # last const DMA gates the first xbar transpose

            def emit_front(j, b0, blk):
                """mm1 + tanh + score matvecs + transposes + softmax for block j.
                Returns (axn_slices, wmt) needed by the deferred out matvecs."""
                score_ps = spsp.tile([S, blk], f32, tag="score", name=f"score{j}")
                axn_slices = []
                ZG = 2 * GROUP
                for g in range(blk // ZG):
                    gb = b0 + g * ZG
                    tok0 = gb * S
                    if (g * ZG) % (XLOAD * GROUP) == 0:
                        if j == 0 and g == 0:
                            emit_front.xt_sb = xpre_sb
                        else:
                            xt_sb = xinp.tile([D, XLOAD * GROUP * S], bf16,
                                              tag="xt", name=f"xt{j}_{g}")
                            if variant == "dmat":
                                nc.sync.dma_start_transpose(
                                    xt_sb, xd[tok0:tok0 + XLOAD * GROUP * S, :])
                            else:
                                nc.sync.dma_start(
                                    xt_sb, xd[:, tok0:tok0 + XLOAD * GROUP * S])
                            emit_front.xt_sb = xt_sb
                    xt_sb = emit_front.xt_sb
                    xoff = (g * ZG * S) % (XLOAD * GROUP * S)
                    z_ps = zpsp.tile([H, ZG * S], f32, tag="z", name=f"z{j}_{g}")
                    for h in range(ZG // GROUP):
                        nc.tensor.matmul(
                            z_ps[:, h * GROUP * S:(h + 1) * GROUP * S], wat_sb,
                            xt_sb[:, xoff + h * GROUP * S:xoff + (h + 1) * GROUP * S],
                            start=True, stop=True)
                    axt_sb = axtp.tile([H, ZG * S], bf16, tag="axt",
                                       name=f"axt{j}_{g}")
                    nc.scalar.activation(axt_sb, z_ps, AF.Tanh, bias=battr_sb)

                    if variant == "pe":
                        axn_ps = tpsp.tile([S, ZG * H], bf16, tag="axn_ps",
                                           name=f"axn_ps{j}_{g}")
                    axn_sb = axnp.tile([S, ZG * H], bf16, tag="axn",
                                       name=f"axn{j}_{g}")
                    for i in range(ZG):
                        b = gb + i
                        bi = b - b0
                        a_slice = axt_sb[:, i * S:(i + 1) * S]
                        nc.tensor.matmul(
                            score_ps[:, bi:bi + 1], a_slice, rt_sb[:, b:b + 1],
                            start=True, stop=True,
                        )
                        if variant == "pe":
                            nc.tensor.transpose(
                                axn_ps[:, i * H:(i + 1) * H], a_slice, ident)
                        else:
                            nc.sync.dma_start_transpose(
                                axn_sb[:, i * H:(i + 1) * H], a_slice)
                    if variant == "pe":
                        nc.vector.tensor_copy(axn_sb, axn_ps)
                    for i in range(ZG):
                        axn_slices.append(axn_sb[:, i * H:(i + 1) * H])

                # softmax in transposed layout [S, BLK]
                m_slice = maskt_sb[:, b0:b0 + blk]
                smt = softp.tile([S, blk], f32, tag="smt", name=f"smt{j}")
                nc.vector.tensor_mul(smt, score_ps, m_slice)
                expt = softp.tile([S, blk], f32, tag="expt", name=f"expt{j}")
                nc.scalar.activation(expt, smt, AF.Exp)
                if F32NORM:
                    expt_n = expt
                else:
                    expt_n = softp.tile([S, blk], mybir.dt.bfloat16,
                                        tag="expt_n", name=f"expt_n{j}")
                    nc.vector.tensor_copy(expt_n, expt)
                den_ps = npsp.tile([1, blk], f32, tag="nrm", name=f"den_ps{j}")
                nc.tensor.matmul(den_ps, ones_col, expt_n, start=True, stop=True)
                rden = softp.tile([1, blk], f32, tag="rden", name=f"rden{j}")
                nc.vector.reciprocal(rden, den_ps)
                if F32NORM:
                    rden_n = rden
                else:
                    rden_n = softp.tile([1, blk], mybir.dt.bfloat16,
                                        tag="rden_n", name=f"rden_n{j}")
                    nc.vector.tensor_copy(rden_n, rden)
                rb_ps = npsp.tile([S, blk], f32, tag="nrm", name=f"rb_ps{j}")
                nc.tensor.matmul(rb_ps, ones_row, rden_n, start=True, stop=True)
                awt_slice = outwt_sb[:, b0:b0 + blk]
                nc.vector.tensor_mul(awt_slice, expt, rb_ps)
                wmt = softp.tile([S, blk], bf16, tag="wmt", name=f"wmt{j}")
                nc.vector.tensor_mul(wmt, awt_slice, m_slice)
                return axn_slices, wmt

            def emit_out(j, b0, blk, axn_slices, wmt):
                out_ps = opsp.tile([H, blk], f32, tag="out", name=f"out{j}")
                for bi in range(blk):
                    nc.tensor.matmul(
                        out_ps[:, bi:bi + 1], axn_slices[bi], wmt[:, bi:bi + 1],
                        start=True, stop=True,
                    )
                nc.vector.tensor_copy(outxt_sb[:, b0:b0 + blk], out_ps)

            # software pipeline: block j's out matvecs are emitted after
            # block j+1's scores, so the PE never waits on a softmax chain
            blocks = [(j * BLK, BLK) for j in range(NB // BLK - 1)]
            last = (NB // BLK - 1) * BLK
            blocks += [(last, BLK // 2), (last + BLK // 2, BLK // 2)]
            pending = None
            for j, (b0, blk) in enumerate(blocks):
                front = emit_front(j, b0, blk)
                if pending is not None:
                    emit_out(*pending)
                pending = (j, b0, blk, front[0], front[1])
            emit_out(*pending)

            nc.sync.dma_start(outwd[:, :], outwt_sb)
            nc.sync.dma_start(outxd[:, :], outxt_sb)

    nc.compile()
    return nc


def _host_prep(x, x_mask, user, user_size, W_attr, b_attr, W_user, b_user,
               variant=None):
    variant = variant or VARIANT
    x = np.asarray(x)
    x_mask = np.asarray(x_mask, dtype=np.float32)
    user = np.asarray(user, dtype=np.float32)
    user_size = np.asarray(user_size)
    W_attr = np.asarray(W_attr, dtype=np.float32)
    b_attr = np.asarray(b_attr, dtype=np.float32)
    W_user = np.asarray(W_user, dtype=np.float32)
    b_user = np.asarray(b_user, dtype=np.float32)

    # user projection + repeat_interleave gather (tiny; jnp.repeat
    # total_repeat_length semantics: truncate, or pad with final value)
    attn_user = user @ W_user.T + b_user               # [U, H]
    reps = np.repeat(np.arange(U), np.maximum(user_size.astype(np.int64), 0))
    if reps.size >= B:
        idx = reps[:B]
    else:
        pad_val = reps[-1] if reps.size else 0
        idx = np.concatenate([reps, np.full(B - reps.size, pad_val, dtype=np.int64)])
    R = attn_user[idx]                                  # [B, H] f32

    wat_np = np.ascontiguousarray(W_attr.T).astype(np.float16)
    battr_np = np.ascontiguousarray(b_attr.reshape(H, 1))

    in_maps = []
    for c in range(NCORES):
        sl = slice(c * NB, (c + 1) * NB)
        xs = x[sl].astype(np.float16)           # [NB, S, D]
        if variant == "dmat":
            x_np = np.ascontiguousarray(xs.reshape(NB * S, D))
        else:
            x_np = np.ascontiguousarray(xs.reshape(NB * S, D).T)
        maskt_np = np.ascontiguousarray(x_mask[sl].T)
        rt_np = np.ascontiguousarray(R[sl].T.astype(np.float16))
        in_maps.append({
            "xd": x_np, "masktd": maskt_np, "rtd": rt_np,
            "watd": wat_np, "battrd": battr_np,
        })
    return in_maps


def kernel(x, x_mask, user, user_size, W_attr, b_attr, W_user, b_user):
    global LAST_RESULTS
    from concourse.bass_utils import run_bass_kernel_spmd

    in_maps = _host_prep(x, x_mask, user, user_size, W_attr, b_attr,
                         W_user, b_user)
    if VARIANT not in _compiled:
        _compiled[VARIANT] = _build_bass()
    LAST_RESULTS = run_bass_kernel_spmd(
        _compiled[VARIANT], in_maps, core_ids=list(range(NCORES)), trace=TRACE,
    )
    outs = LAST_RESULTS.results
    attn_weighted_x = np.concatenate(
        [np.ascontiguousarray(o["outxd"].T) for o in outs], axis=0)
    attn_weight = np.concatenate(
        [np.ascontiguousarray(o["outwd"].T) for o in outs], axis=0)
    return attn_weighted_x, attn_weight


# revision 24
# speedup vs baseline: 6.7532x; 1.1849x over previous
"""Trainium2 Bass kernel for the ATTR_TRANSFORMER pooling module.

Computation (per batch row b):
    attn_x  = tanh(x[b] @ W_attr.T + b_attr)            # [S, H]
    ru      = (user @ W_user.T + b_user)[idx[b]]        # [H]
    score   = (attn_x * mask[b,:,None]) @ ru            # [S]
    w       = softmax(score)                            # [S]
    out_x   = attn_x.T @ (w * mask[b])                  # [H]

Sharding: data-parallel over B across 8 NeuronCores (512 rows each).
Host prep: the tiny [U,H] user projection + repeat_interleave gather
(~0.05% of FLOPs) and layout marshalling (transposes/casts of inputs,
un-transpose of outputs).

Device pipeline per core, built around two hardware constraints —
matmul PSUM outputs may only start at partition 0/32/64, and per-batch
matvecs are cheapest with the attn tile stationary — so all per-batch
results are produced as PSUM *columns* (free-dim offsets are
unrestricted) and the softmax runs entirely in the transposed
[S, batch] layout:
  - mm1:  stationary W_attr.T [D,H] bf16, moving x.T [D, 4*S] -> z.T [H,512]
  - tanh on ACT with per-partition bias b_attr -> attn_x.T [H, 4*S] bf16
  - score matvec: lhsT = attn_x.T[b] [H,S] (FWL), rhs = ru[b] [H,1]
    -> score.T column -> PSUM [S, BLK]
  - softmax without max-subtraction (scores are O(10), exp is f32-safe):
    mask-mul (DVE) -> exp (ACT) -> den = ones.T @ expT on PE [1,BLK] ->
    reciprocal (DVE) -> broadcast via K=1 ones matmul [S,BLK] ->
    normalize + re-mask (DVE)
  - attn_x.T[b] transposed to natural [S,H] per batch (PE+identity by
    default; xbar DMA-transpose variant behind KVARIANT=dmat)
  - out matvec: lhsT = attn_x[b] [S,H], rhs = (w*mask).T[:,b] [S,1]
    -> out.T column -> PSUM [H, BLK]
Outputs are staged transposed ([H,NB], [S,NB]) and un-transposed on host.
"""

import os

import numpy as np


B, S, D, H, U = 4096, 128, 128, 128, 256
NCORES = 8
NB = B // NCORES          # 512 batches per core
GROUP = 4                 # batches per mm1 matmul (moving N = 512)
XLOAD = 4                 # mm1 groups per x DMA (512 KB loads)
BLK = 128                 # batches per softmax block

# "pe":   attn transposes on the tensor engine (no xbar DMA transposes)
# "dmat": attn transposes + x loads via xbar DMA transpose
VARIANT = os.environ.get("KVARIANT", "pe")
F32NORM = os.environ.get("KF32NORM", "1") == "1"  # f32 ones-matmuls for softmax norm

TRACE = False
LAST_RESULTS = None

_compiled = {}


def _build_bass(variant=None):
    import concourse.bacc as bacc
    import concourse.tile as tile
    from concourse import mybir
    from concourse.masks import make_identity

    variant = variant or VARIANT
    f32 = mybir.dt.float32
    bf16 = mybir.dt.float16  # fp16 compute: 10 mantissa bits vs bf16 8
    AF = mybir.ActivationFunctionType
    ALU = mybir.AluOpType
    norm_dt = f32 if F32NORM else bf16

    nc = bacc.Bacc("TRN2", target_bir_lowering=False, debug=False)

    if variant == "dmat":
        xd = nc.dram_tensor("xd", [NB * S, D], bf16, kind="ExternalInput")
    else:
        xd = nc.dram_tensor("xd", [D, NB * S], bf16, kind="ExternalInput")
    masktd = nc.dram_tensor("masktd", [S, NB], f32, kind="ExternalInput")
    rtd = nc.dram_tensor("rtd", [H, NB], bf16, kind="ExternalInput")
    watd = nc.dram_tensor("watd", [D, H], bf16, kind="ExternalInput")
    battrd = nc.dram_tensor("battrd", [H, 1], f32, kind="ExternalInput")
    outxd = nc.dram_tensor("outxd", [H, NB], f32, kind="ExternalOutput")
    outwd = nc.dram_tensor("outwd", [S, NB], f32, kind="ExternalOutput")

    with tile.TileContext(nc) as tc:
        with (
            tc.tile_pool(name="const", bufs=1) as constp,
            tc.tile_pool(name="xin", bufs=6) as xinp,
            tc.tile_pool(name="axt", bufs=8) as axtp,
            tc.tile_pool(name="axn", bufs=(BLK // GROUP)) as axnp,  # [S, ZG*H] tiles x 2 blocks
            tc.tile_pool(name="soft", bufs=3) as softp,
            tc.tile_pool(name="zps", bufs=2, space="PSUM") as zpsp,
            tc.tile_pool(name="sps", bufs=1, space="PSUM") as spsp,
            tc.tile_pool(name="nps", bufs=1, space="PSUM") as npsp,
            tc.tile_pool(name="ops", bufs=1, space="PSUM") as opsp,
            tc.tile_pool(name="tps", bufs=1, space="PSUM") as tpsp,
        ):
            wat_sb = constp.tile([D, H], bf16)
            nc.gpsimd.dma_start(wat_sb, watd[:, :])
            battr_sb = constp.tile([H, 1], f32)
            nc.gpsimd.dma_start(battr_sb, battrd[:, :])
            rt_sb = constp.tile([H, NB], bf16)
            nc.gpsimd.dma_start(rt_sb, rtd[:, :])
            # prefetch block 0's first x tile before the bulky mask DMA so the
            # PE pipeline ramps immediately
            xpre_sb = constp.tile([D, XLOAD * GROUP * S], bf16, name="xpre")
            if variant == "dmat":
                nc.sync.dma_start_transpose(xpre_sb, xd[0:XLOAD * GROUP * S, :])
            else:
                nc.sync.dma_start(xpre_sb, xd[:, 0:XLOAD * GROUP * S])
            maskt_sb = constp.tile([S, NB], f32)
            nc.gpsimd.dma_start(maskt_sb, masktd[:, :])
            warm = constp.tile([1, 1], f32)
            nc.vector.memset(warm, 0.0)
            nc.scalar.activation(warm, warm, AF.Tanh)  # prefetch ACT table set
            ones_dt = f32 if F32NORM else mybir.dt.bfloat16
            ones_col = constp.tile([S, 1], ones_dt)
            nc.vector.memset(ones_col, 1.0)
            ones_row = constp.tile([1, S], ones_dt)
            nc.vector.memset(ones_row, 1.0)
            if variant == "pe":
                ident = constp.tile([128, 128], bf16)
                make_identity(nc, ident)
            outwt_sb = constp.tile([S, NB], f32)
            outxt_sb = constp.tile([H, NB], f32)
            const_dmas = []  # last const DMA gates the first xbar transpose

            def emit_front(j, b0, blk):
                """mm1 + tanh + score matvecs + transposes + softmax for block j.
                Returns (axn_slices, wmt) needed by the deferred out matvecs."""
                score_ps = spsp.tile([S, blk], f32, tag="score", name=f"score{j}")
                axn_slices = []
                ZG = 2 * GROUP
                for g in range(blk // ZG):
                    gb = b0 + g * ZG
                    tok0 = gb * S
                    if (g * ZG) % (XLOAD * GROUP) == 0:
                        if j == 0 and g == 0:
                            emit_front.xt_sb = xpre_sb
                        else:
                            xt_sb = xinp.tile([D, XLOAD * GROUP * S], bf16,
                                              tag="xt", name=f"xt{j}_{g}")
                            if variant == "dmat":
                                nc.sync.dma_start_transpose(
                                    xt_sb, xd[tok0:tok0 + XLOAD * GROUP * S, :])
                            else:
                                nc.sync.dma_start(
                                    xt_sb, xd[:, tok0:tok0 + XLOAD * GROUP * S])
                            emit_front.xt_sb = xt_sb
                    xt_sb = emit_front.xt_sb
                    xoff = (g * ZG * S) % (XLOAD * GROUP * S)
                    z_ps = zpsp.tile([H, ZG * S], f32, tag="z", name=f"z{j}_{g}")
                    for h in range(ZG // GROUP):
                        nc.tensor.matmul(
                            z_ps[:, h * GROUP * S:(h + 1) * GROUP * S], wat_sb,
                            xt_sb[:, xoff + h * GROUP * S:xoff + (h + 1) * GROUP * S],
                            start=True, stop=True)
                    axt_sb = axtp.tile([H, ZG * S], bf16, tag="axt",
                                       name=f"axt{j}_{g}")
                    nc.scalar.activation(axt_sb, z_ps, AF.Tanh, bias=battr_sb)

                    if variant == "pe":
                        axn_ps = tpsp.tile([S, ZG * H], bf16, tag="axn_ps",
                                           name=f"axn_ps{j}_{g}")
                    axn_sb = axnp.tile([S, ZG * H], bf16, tag="axn",
                                       name=f"axn{j}_{g}")
                    for i in range(ZG):
                        b = gb + i
                        bi = b - b0
                        a_slice = axt_sb[:, i * S:(i + 1) * S]
                        nc.tensor.matmul(
                            score_ps[:, bi:bi + 1], a_slice, rt_sb[:, b:b + 1],
                            start=True, stop=True,
                        )
                        if variant == "pe":
                            nc.tensor.transpose(
                                axn_ps[:, i * H:(i + 1) * H], a_slice, ident)
                        else:
                            nc.sync.dma_start_transpose(
                                axn_sb[:, i * H:(i + 1) * H], a_slice)
                    if variant == "pe":
                        nc.vector.tensor_copy(axn_sb, axn_ps)
                    for i in range(ZG):
                        axn_slices.append(axn_sb[:, i * H:(i + 1) * H])

                # softmax in transposed layout [S, BLK]
                m_slice = maskt_sb[:, b0:b0 + blk]
                smt = softp.tile([S, blk], f32, tag="smt", name=f"smt{j}")
                nc.vector.tensor_mul(smt, score_ps, m_slice)
                expt = softp.tile([S, blk], f32, tag="expt", name=f"expt{j}")
                nc.scalar.activation(expt, smt, AF.Exp)
                if F32NORM:
                    expt_n = expt
                else:
                    expt_n = softp.tile([S, blk], mybir.dt.bfloat16,
                                        tag="expt_n", name=f"expt_n{j}")
                    nc.vector.tensor_copy(expt_n, expt)
                den_ps = npsp.tile([1, blk], f32, tag="nrm", name=f"den_ps{j}")
                nc.tensor.matmul(den_ps, ones_col, expt_n, start=True, stop=True)
                rden = softp.tile([1, blk], f32, tag="rden", name=f"rden{j}")
                nc.vector.reciprocal(rden, den_ps)
                if F32NORM:
                    rden_n = rden
                else:
                    rden_n = softp.tile([1, blk], mybir.dt.bfloat16,
                                        tag="rden_n", name=f"rden_n{j}")
                    nc.vector.tensor_copy(rden_n, rden)
                rb_ps = npsp.tile([S, blk], f32, tag="nrm", name=f"rb_ps{j}")
                nc.tensor.matmul(rb_ps, ones_row, rden_n, start=True, stop=True)
                awt_slice = outwt_sb[:, b0:b0 + blk]
                nc.vector.tensor_mul(awt_slice, expt, rb_ps)
                wmt = softp.tile([S, blk], bf16, tag="wmt", name=f"wmt{j}")
                nc.vector.tensor_mul(wmt, awt_slice, m_slice)
                return axn_slices, wmt

            def emit_out(j, b0, blk, axn_slices, wmt):
                out_ps = opsp.tile([H, blk], f32, tag="out", name=f"out{j}")
                for bi in range(blk):
                    nc.tensor.matmul(
                        out_ps[:, bi:bi + 1], axn_slices[bi], wmt[:, bi:bi + 1],
                        start=True, stop=True,
                    )
                nc.vector.tensor_copy(outxt_sb[:, b0:b0 + blk], out_ps)

            # software pipeline: block j's out matvecs are emitted after
            # block j+1's scores, so the PE never waits on a softmax chain
            blocks = [(j * BLK, BLK) for j in range(NB // BLK - 1)]
            last = (NB // BLK - 1) * BLK
            blocks += [(last, BLK // 2), (last + BLK // 2, BLK // 2)]
            pending = None
            for j, (b0, blk) in enumerate(blocks):
                front = emit_front(j, b0, blk)
                if pending is not None:
                    emit_out(*pending)
                pending = (j, b0, blk, front[0], front[1])
            emit_out(*pending)

            nc.sync.dma_start(outwd[:, :], outwt_sb)
            nc.sync.dma_start(outxd[:, :], outxt_sb)

    nc.compile()
    return nc


def _host_prep(x, x_mask, user, user_size, W_attr, b_attr, W_user, b_user,
               variant=None):
    variant = variant or VARIANT
    x = np.asarray(x)
    x_mask = np.asarray(x_mask, dtype=np.float32)
    user = np.asarray(user, dtype=np.float32)
    user_size = np.asarray(user_size)
    W_attr = np.asarray(W_attr, dtype=np.float32)
    b_attr = np.asarray(b_attr, dtype=np.float32)
    W_user = np.asarray(W_user, dtype=np.float32)
    b_user = np.asarray(b_user, dtype=np.float32)

    # user projection + repeat_interleave gather (tiny; jnp.repeat
    # total_repeat_length semantics: truncate, or pad with final value)
    attn_user = user @ W_user.T + b_user               # [U, H]
    reps = np.repeat(np.arange(U), np.maximum(user_size.astype(np.int64), 0))
    if reps.size >= B:
        idx = reps[:B]
    else:
        pad_val = reps[-1] if reps.size else 0
        idx = np.concatenate([reps, np.full(B - reps.size, pad_val, dtype=np.int64)])
    R = attn_user[idx]                                  # [B, H] f32

    wat_np = np.ascontiguousarray(W_attr.T).astype(np.float16)
    battr_np = np.ascontiguousarray(b_attr.reshape(H, 1))

    in_maps = []
    for c in range(NCORES):
        sl = slice(c * NB, (c + 1) * NB)
        xs = x[sl].astype(np.float16)           # [NB, S, D]
        if variant == "dmat":
            x_np = np.ascontiguousarray(xs.reshape(NB * S, D))
        else:
            x_np = np.ascontiguousarray(xs.reshape(NB * S, D).T)
        maskt_np = np.ascontiguousarray(x_mask[sl].T)
        rt_np = np.ascontiguousarray(R[sl].T.astype(np.float16))
        in_maps.append({
            "xd": x_np, "masktd": maskt_np, "rtd": rt_np,
            "watd": wat_np, "battrd": battr_np,
        })
    return in_maps


def kernel(x, x_mask, user, user_size, W_attr, b_attr, W_user, b_user):
    global LAST_RESULTS
    from concourse.bass_utils import run_bass_kernel_spmd

    in_maps = _host_prep(x, x_mask, user, user_size, W_attr, b_attr,
                         W_user, b_user)
    if VARIANT not in _compiled:
        _compiled[VARIANT] = _build_bass()
    LAST_RESULTS = run_bass_kernel_spmd(
        _compiled[VARIANT], in_maps, core_ids=list(range(NCORES)), trace=TRACE,
    )
    outs = LAST_RESULTS.results
    attn_weighted_x = np.concatenate(
        [np.ascontiguousarray(o["outxd"].T) for o in outs], axis=0)
    attn_weight = np.concatenate(
        [np.ascontiguousarray(o["outwd"].T) for o in outs], axis=0)
    return attn_weighted_x, attn_weight
